# revision 4
# baseline (speedup 1.0000x reference)
"""Chamfer distance kernel for Trainium2 (8 NeuronCores, batch-parallel).

Problem: input1 (8,4096,3), input2 (8,4096,3) fp32.
  D[b,n,m] = ||input1[b,n]-input2[b,m]||
  loss = mean_b( mean_m min_n D + mean_n min_m D )

Per core (one batch): -D2 = 2*x1.x2 - n1[n] - n2[m] computed on the PE as a
single K=13 float32r matmul whose contraction rows carry the hi/lo split of
the coordinates plus the hi/lo split of both squared norms (the hi+lo f32r
pair reconstructs fp32 exactly, so D2 is fp32-accurate up to the dropped
lo*lo term ~2^-26). The sign is flipped so both reductions are MAX. Each
4-bank PSUM group (128x2048) is copied once by the Scalar engine to bf16
SBUF; from that copy the Vector engine accumulates column maxes elementwise
(bf16 tensor_tensor is the fastest DVE op) and computes row maxes by
pairwise-max halving (tensor_reduce is stuck at 1 elem/lane/cycle). Column
maxes are reduced across partitions with gpsimd.partition_all_reduce(max).
sqrt only touches the 2*4096 winning mins: sqrt(-x) via the activation
scale, after clamping (cancellation can leave the smallest D2 at ~-5e-7).
Host averages the per-core sums (the batch mean is the unshard step).
"""

import sys

sys.path.insert(0, "/opt/trn_rl_repo")

import numpy as np
from contextlib import ExitStack

import concourse.bacc as bacc
import concourse.tile as tile
import concourse.bass_isa as bass_isa
from concourse import mybir
from concourse.bass_utils import run_bass_kernel_spmd

B, NPTS, KDIM = 8, 4096, 3
IT_N = NPTS // 128   # 32 I-tiles of 128 rows (x1 points)
JC_N = NPTS // 512   # 8 J-chunks of 512 cols (x2 points)

F32 = mybir.dt.float32
F32R = mybir.dt.float32r

_cached = {}


def _build(reps: int = 1, loop_n: int = 1, GSPAN: int = 2048, PSB: int = 2, CBB: int = 3, HYB: int = 0):
    nc = bacc.Bacc("TRN2", target_bir_lowering=False, debug=False, num_devices=B)

    x1_d = nc.dram_tensor("x1", [NPTS, KDIM], F32, kind="ExternalInput").ap()
    x2_d = nc.dram_tensor("x2", [NPTS, KDIM], F32, kind="ExternalInput").ap()
    outc_d = nc.dram_tensor("outc", [128, IT_N], F32, kind="ExternalOutput").ap()
    outr_d = nc.dram_tensor("outr", [128, IT_N], F32, kind="ExternalOutput").ap()

    MX = mybir.AluOpType.max
    X = mybir.AxisListType.X

    with tile.TileContext(nc) as tc, ExitStack() as ctx:
        sb = ctx.enter_context(tc.tile_pool(name="sb", bufs=1))
        scr = ctx.enter_context(tc.tile_pool(name="scr", bufs=6))
        stg = ctx.enter_context(tc.tile_pool(name="stg", bufs=1))
        cbp = ctx.enter_context(tc.tile_pool(name="cbp", bufs=CBB))
        tsp = ctx.enter_context(tc.tile_pool(name="tsp", bufs=2))
        ps = ctx.enter_context(tc.tile_pool(name="ps", bufs=PSB, space="PSUM"))

        # Engine SBUF ops must start at partition 0/32/64/96, so the 13-row
        # operands are staged in fp32 via DMA (any partition base), then
        # rounded to f32r in one 13-partition copy. That copy turns the raw
        # rows into their `hi` parts; the `lo` rows were computed as
        # x - f32r(x), which f32r represents exactly.
        # P = sum_r L[r]*R[r] = 2*x1.x2 - n1 - n2 = -D2 (float32r limbs:
        # hi+lo reconstructs fp32 exactly, so D2 is fp32-accurate up to the
        # dropped lo*lo term ~2^-26):
        # r    L row         R row
        # 0-2  x1hi          2*x2hi
        # 3-5  x1hi          2*x2lo
        # 6-8  x1lo          2*x2hi
        # 9    n1hi          -1
        # 10   n1lo          -1
        # 11   +1            -n2hi
        # 12   +1            -n2lo
        BF16 = mybir.dt.bfloat16
        KROWS = 13
        L = sb.tile([KROWS, NPTS], F32R)
        R = sb.tile([KROWS, NPTS], F32R)

        # All per-point math runs in natural layout (128, 32, 3) so every DVE
        # lane works (the (3, NPTS) layout would idle 125/128 lanes); results
        # are scattered into the staging rows by DMA. Column order of L/R is
        # point index n = p*32 + t in both layouts, so no permutation arises.
        def row_view(S, k):
            # (1, 4096) staging row as (1, 128, 32) iterating (p, t)
            return S[k : k + 1, :].rearrange("o (p t) -> o p t", p=128)

        def stage_side(S, x_d, scale, norm_factor, hi_rows_extra, lo_rows, n_rows, ones_rows, const_nat):
            xn = scr.tile([128, 96], F32, tag="nat")
            nc.sync.dma_start(xn[:], x_d.rearrange("(p t) k -> p (t k)", p=128))
            if scale != 1.0:
                nc.vector.tensor_scalar_mul(xn[:], xn[:], scale)
            xnv = xn[:].rearrange("p (t k) -> p t k", k=KDIM)
            # norm = norm_factor/scale^2 * sum_k (scale*x_k)^2
            sqn = scr.tile([128, 96], F32, tag="nat")
            nc.scalar.square(sqn[:], xn[:])
            nn = scr.tile([128, 32], F32, tag="natn")
            nc.vector.tensor_reduce(
                nn[:], sqn[:].rearrange("p (t k) -> p t k", k=KDIM), axis=X,
                op=mybir.AluOpType.add,
            )
            f = norm_factor / (scale * scale)
            if f != 1.0:
                nc.vector.tensor_scalar_mul(nn[:], nn[:], f)
            # hi/lo splits (lo = x - f32r(x) is exactly representable in f32r;
            # the final f32r copy of S rounds the raw rows to their hi limbs)
            hin = scr.tile([128, 96], F32R, tag="nat")
            nc.vector.tensor_copy(hin[:], xn[:])
            lon = scr.tile([128, 96], F32, tag="nat")
            nc.vector.tensor_sub(lon[:], xn[:], hin[:].bitcast(F32))
            lonv = lon[:].rearrange("p (t k) -> p t k", k=KDIM)
            nhn = scr.tile([128, 32], F32R, tag="natn")
            nc.vector.tensor_copy(nhn[:], nn[:])
            nln = scr.tile([128, 32], F32, tag="natn")
            nc.vector.tensor_sub(nln[:], nn[:], nhn[:].bitcast(F32))
            for k in range(KDIM):
                nc.sync.dma_start(row_view(S, k), xnv[:, :, k])
                if hi_rows_extra is not None:
                    nc.sync.dma_start(row_view(S, hi_rows_extra + k), xnv[:, :, k])
                else:
                    nc.sync.dma_start(row_view(S, 3 + k), xnv[:, :, k])
                nc.sync.dma_start(row_view(S, lo_rows + k), lonv[:, :, k])
            nc.sync.dma_start(row_view(S, n_rows), nn[:])
            nc.sync.dma_start(row_view(S, n_rows + 1), nln[:])
            # constant rows: source order is irrelevant for a constant fill
            nc.sync.dma_start(
                S[ones_rows[0] : ones_rows[1], :], const_nat[:, : (ones_rows[1] - ones_rows[0]) * 32]
            )

        ones_nat = scr.tile([128, 64], F32, tag="natc")
        nc.vector.memset(ones_nat[:], 1.0)
        mones_nat = scr.tile([128, 64], F32, tag="natc")
        nc.vector.memset(mones_nat[:], -1.0)

        S1 = stg.tile([KROWS, NPTS], F32, tag="stage")
        stage_side(S1, x1_d, 1.0, 1.0, None, 6, 9, (11, 13), ones_nat)
        nc.vector.tensor_copy(L[:], S1[:])

        S2 = stg.tile([KROWS, NPTS], F32, tag="stage")
        stage_side(S2, x2_d, 2.0, -1.0, 6, 3, 11, (9, 11), mones_nat)
        nc.vector.tensor_copy(R[:], S2[:])

        # ping-pong accumulators: out != in0 keeps the bf16 tensor_tensor in
        # its 2x perf mode (in-place aliasing falls back to 1x)
        cmb_a = sb.tile([128, NPTS], BF16)
        cmb_b = sb.tile([128, NPTS], BF16)
        nc.vector.memset(cmb_a[:], -3.0e38)
        rmall = sb.tile([128, IT_N], F32)

        # ---- main loop: -D2 tiles on PE (4x512 into a 4-bank PSUM group),
        # one ACT copy fp32->bf16 per group, then DVE: colmax accumulate
        # (bf16 tensor_tensor, 2x mode) + rowmax in ONE tensor_scalar pass
        # whose accum_out reduces the row with op1=max (4x mode — the only
        # DVE op family that both reduces and runs 4x) ----
        # (reps/loop_n repeat the identical main loop for differential HW timing)
        GRP = GSPAN // 512  # jc chunks per PSUM group
        NG = JC_N // GRP   # groups per I-tile
        rg_all = sb.tile([128, IT_N * NG], F32)
        import contextlib
        loop_ctx = tc.For_i(0, loop_n, 1) if loop_n > 1 else contextlib.nullcontext()
        with loop_ctx:
          for _rep in range(reps):
            for it in range(IT_N):
                for g in range(NG):
                    P = ps.tile([128, GSPAN], F32)
                    for j in range(GRP):
                        nc.tensor.matmul(
                            P[:, j * 512 : (j + 1) * 512],
                            L[:, it * 128 : (it + 1) * 128],
                            R[:, (g * GRP + j) * 512 : (g * GRP + j + 1) * 512],
                            start=True,
                            stop=True,
                        )
                    src, dst = (cmb_a, cmb_b) if it % 2 == 0 else (cmb_b, cmb_a)
                    sl = slice(g * GSPAN, (g + 1) * GSPAN)
                    C = cbp.tile([128, GSPAN], BF16)
                    nc.scalar.copy(C[:], P[:])
                    nc.vector.tensor_tensor(dst[:, sl], src[:, sl], C[:], op=MX)
                    ts = tsp.tile([128, GSPAN], BF16)
                    nc.vector.tensor_scalar(
                        ts[:], C[:], -3.0e38, None, op0=MX, op1=MX,
                        accum_out=rg_all[:, it * NG + g : it * NG + g + 1],
                    )
        # fold the NG per-group rowmaxes into one column per I-tile
        rga_v = rg_all[:].rearrange("p (t g) -> p t g", g=NG)
        nc.vector.tensor_tensor(rmall[:], rga_v[:, :, 0], rga_v[:, :, 1], op=MX)
        for g in range(2, NG):
            nc.vector.tensor_tensor(rmall[:], rmall[:], rga_v[:, :, g], op=MX)

        # ---- tail: partition-max of cmb on gpsimd, then gather row 0 into
        # natural (128, 32) layout by DMA so the clamp/sqrt use all lanes ----
        cmb_fin = cmb_b if (IT_N * reps) % 2 == 1 else cmb_a
        cmr = sb.tile([128, NPTS], BF16)
        nc.gpsimd.partition_all_reduce(
            cmr[:], cmb_fin[:], channels=128, reduce_op=bass_isa.ReduceOp.max
        )
        cmd = sb.tile([128, IT_N], BF16)
        nc.sync.dma_start(
            cmd[:], cmr[0:1, :].rearrange("o (p t) -> o p t", p=128)
        )
        nc.vector.tensor_scalar_min(cmd[:], cmd[:], 0.0)
        nc.vector.tensor_scalar_min(rmall[:], rmall[:], 0.0)
        o0 = sb.tile([128, IT_N], F32)
        o1 = sb.tile([128, IT_N], F32)
        nc.scalar.activation(o0[:], cmd[:], mybir.ActivationFunctionType.Sqrt, scale=-1.0)
        nc.scalar.activation(o1[:], rmall[:], mybir.ActivationFunctionType.Sqrt, scale=-1.0)
        nc.sync.dma_start(outc_d[:], o0[:])
        nc.sync.dma_start(outr_d[:], o1[:])

    nc.compile()
    return nc


def _get(reps: int = 1, loop_n: int = 1, **kw):
    key = (reps, loop_n, tuple(sorted(kw.items())))
    if key not in _cached:
        _cached[key] = _build(reps, loop_n, **kw)
    return _cached[key]


def kernel(input1: np.ndarray, input2: np.ndarray, _trace: bool = False):
    nc = _get()
    input1 = np.ascontiguousarray(np.asarray(input1, dtype=np.float32))
    input2 = np.ascontiguousarray(np.asarray(input2, dtype=np.float32))
    in_maps = [{"x1": input1[b], "x2": input2[b]} for b in range(B)]
    res = run_bass_kernel_spmd(nc, in_maps, core_ids=list(range(B)), trace=_trace)
    losses = []
    for b in range(B):
        r = res.results[b]
        losses.append(
            r["outc"].mean(dtype=np.float64) + r["outr"].mean(dtype=np.float64)
        )
    out = np.float32(np.mean(losses))
    if _trace:
        return out, res
    return out



# revision 18
# speedup vs baseline: 3.8483x; 3.8483x over previous
"""Chamfer distance kernel for Trainium2 (8 NeuronCores, batch-parallel).

Problem: input1 (8,4096,3), input2 (8,4096,3) fp32.
  D[b,n,m] = ||input1[b,n]-input2[b,m]||
  loss = mean_b( mean_m min_n D + mean_n min_m D )

Per core (one batch): -D2 = 2*x1.x2 - n1[n] - n2[m] computed on the PE as a
single K=13 float32r matmul whose contraction rows carry the hi/lo split of
the coordinates plus the hi/lo split of both squared norms (the hi+lo f32r
pair reconstructs fp32 exactly, so D2 is fp32-accurate up to the dropped
lo*lo term ~2^-26). The sign is flipped so both reductions are MAX.

Main loop, per 128x2048 PSUM group: the fp32 PSUM tile is converted once to
bf16 SBUF, split ACT[0:CA] / gpsimd[CA:]; the column-min accumulates with a
bf16 tensor_tensor max split DVE[0:CX] / gpsimd[CX:] (gpsimd's chunk runs one
group deferred so its conv chunk never delays the PSUM release); the row-min
comes from ONE DVE tensor_scalar whose accum_out row-reduces with op1=max
(the only DVE op that both reduces and runs in 4x perf mode).

Setup: each side builds a [128, 416] per-point tile (hi limbs pre-rounded to
f32r via copy, lo limbs exact, norms, consts) in 13 column sections ordered
like the L/R staging rows, bounces it to DRAM, and lands it in the [13, 4096]
matmul operand with ONE strided DMA (descriptor-fixed HWDGE cost makes DMA
count, not bytes, the driver). Tail: partition max on gpsimd in two column
halves (the first overlaps the last tiles), raw -D2 minima are DMA'd out and
the host does clamp/sqrt/mean (the batch mean is the unshard step).
"""

import sys

sys.path.insert(0, "/opt/trn_rl_repo")

import numpy as np
from contextlib import ExitStack

import concourse.bacc as bacc
import concourse.tile as tile
import concourse.bass_isa as bass_isa
from concourse import mybir
from concourse.bass_utils import run_bass_kernel_spmd

B, NPTS, KDIM = 8, 4096, 3
IT_N = NPTS // 128   # 32 I-tiles of 128 rows (x1 points)
JC_N = NPTS // 512   # 8 J-chunks of 512 cols (x2 points)

F32 = mybir.dt.float32
F32R = mybir.dt.float32r

_cached = {}


def _build(reps: int = 1, loop_n: int = 1, GSPAN: int = 2048, PSB: int = 2, CBB: int = 3, CA: int = 2048):
    nc = bacc.Bacc("TRN2", target_bir_lowering=False, debug=False, num_devices=B)

    BF16 = mybir.dt.bfloat16
    KROWS = 13
    SECW = KROWS * 32  # 416: one 32-col section per staging row

    x1_d = nc.dram_tensor("x1", [NPTS, KDIM], F32, kind="ExternalInput").ap()
    x2_d = nc.dram_tensor("x2", [NPTS, KDIM], F32, kind="ExternalInput").ap()
    outc_d = nc.dram_tensor("outc", [128, NPTS], BF16, kind="ExternalOutput").ap()
    outr_d = nc.dram_tensor("outr", [128, IT_N], F32, kind="ExternalOutput").ap()
    scr1_d = nc.dram_tensor("scr1", [128 * SECW], F32, kind="Internal").ap()
    scr2_d = nc.dram_tensor("scr2", [128 * SECW], F32, kind="Internal").ap()

    MX = mybir.AluOpType.max
    MUL = mybir.AluOpType.mult
    X = mybir.AxisListType.X

    with tile.TileContext(nc) as tc, ExitStack() as ctx:
        sb = ctx.enter_context(tc.tile_pool(name="sb", bufs=1))
        scr = ctx.enter_context(tc.tile_pool(name="scr", bufs=2))
        cbp = ctx.enter_context(tc.tile_pool(name="cbp", bufs=CBB))
        tsp = ctx.enter_context(tc.tile_pool(name="tsp", bufs=2))
        ps = ctx.enter_context(tc.tile_pool(name="ps", bufs=PSB, space="PSUM"))

        # P = sum_r L[r]*R[r] = 2*x1.x2 - n1 - n2 = -D2 (float32r limbs:
        # hi+lo reconstructs fp32 exactly, so D2 is fp32-accurate up to the
        # dropped lo*lo term ~2^-26):
        # r    L row         R row
        # 0-2  x1hi          2*x2hi
        # 3-5  x1hi          2*x2lo
        # 6-8  x1lo          2*x2hi
        # 9    n1hi          -1
        # 10   n1lo          -1
        # 11   +1            -n2hi
        # 12   +1            -n2lo
        L = sb.tile([KROWS, NPTS], F32R)
        R = sb.tile([KROWS, NPTS], F32R)

        # Per-point math runs in natural layout (128 partitions x 32 points)
        # so every DVE lane works. comb's 13 column sections mirror the L/R
        # rows; section s lands in row s via the single strided DMA below.
        # Column order of L/R is point index n = p*32 + t everywhere.
        def stage_side(x_d, scale, norm_factor, hi_secs, lo_sec, nhi_sec,
                       const_sec, const_val, scratch_d, T, dma):
            xn = scr.tile([128, 96], F32, tag="nat")
            dma.dma_start(xn[:], x_d.rearrange("(p t) k -> p (t k)", p=128))
            # de-interleave (t k) -> (k t), fusing the *2 scale for x2
            xsep = scr.tile([128, 96], F32, tag="natsep")
            osep = xsep[:].rearrange("p (k t) -> p k t", t=32)
            isep = xn[:].rearrange("p (t k) -> p k t", k=KDIM)
            if scale != 1.0:
                nc.vector.tensor_scalar_mul(osep, isep, scale)
            else:
                nc.vector.tensor_copy(osep, isep)
            comb = scr.tile([128, SECW], F32, tag="comb")
            # hi limbs: the f32r-typed copy rounds; the stored bits are both
            # valid f32 and exactly what the PE reads as f32r
            for s in hi_secs:
                nc.vector.tensor_copy(
                    comb[:, s * 32 : (s + 3) * 32].bitcast(F32R), xsep[:]
                )
            h0 = hi_secs[0] * 32
            nc.vector.tensor_sub(
                comb[:, lo_sec * 32 : (lo_sec + 3) * 32],
                xsep[:],
                comb[:, h0 : h0 + 96],
            )
            # norms (of the scaled coords), then hi/lo split
            sq = scr.tile([128, 96], F32, tag="natsq")
            nc.vector.tensor_tensor(sq[:], xsep[:], xsep[:], op=MUL)
            nnr = scr.tile([128, 32], F32, tag="natn")
            nc.vector.tensor_reduce(
                nnr[:], sq[:].rearrange("p (k t) -> p t k", t=32), axis=X,
                op=mybir.AluOpType.add,
            )
            f = norm_factor / (scale * scale)
            if f != 1.0:
                nc.vector.tensor_scalar_mul(nnr[:], nnr[:], f)
            nh0 = nhi_sec * 32
            nc.vector.tensor_copy(comb[:, nh0 : nh0 + 32].bitcast(F32R), nnr[:])
            nc.vector.tensor_sub(
                comb[:, nh0 + 32 : nh0 + 64], nnr[:], comb[:, nh0 : nh0 + 32]
            )
            nc.vector.memset(comb[:, const_sec * 32 : (const_sec + 2) * 32], const_val)
            # bounce to DRAM, then land all 13 rows in ONE strided DMA (same
            # queue, so the read is ordered after the write)
            dma.dma_start(scratch_d.rearrange("(p c) -> p c", p=128), comb[:])
            dma.dma_start(
                T[0:KROWS, :].bitcast(F32).rearrange("r (p t) -> r p t", p=128),
                scratch_d.rearrange("(p r t) -> r p t", p=128, r=KROWS),
            )

        # L: hi(0-2), hi(3-5), lo(6-8), n1hi(9), n1lo(10), +1(11,12)
        stage_side(x1_d, 1.0, 1.0, (0, 3), 6, 9, 11, 1.0, scr1_d, L, nc.sync)
        # R: 2x2hi(0-2), 2x2lo(3-5), 2x2hi(6-8), -1(9,10), n2hi(11), n2lo(12)
        stage_side(x2_d, 2.0, -1.0, (0, 6), 3, 11, 9, -1.0, scr2_d, R, nc.scalar)

        # ping-pong accumulators: out != in0 keeps the bf16 tensor_tensor in
        # its 2x perf mode (in-place aliasing falls back to 1x)
        cmb_a = sb.tile([128, NPTS], BF16)
        cmb_b = sb.tile([128, NPTS], BF16)
        nc.gpsimd.memset(cmb_a[:], -3.0e38)
        rmall = sb.tile([128, IT_N], F32)

        GRP = GSPAN // 512  # jc chunks per PSUM group
        NG = JC_N // GRP   # groups per I-tile
        rg_all = sb.tile([128, IT_N * NG], F32)
        import contextlib
        loop_ctx = tc.For_i(0, loop_n, 1) if loop_n > 1 else contextlib.nullcontext()
        # Real-HW engine constraints (neuronxcc BIR verifier): gpsimd cannot
        # access PSUM and cannot run generic tensor ops at all, so the whole
        # reduction lives on ACT (conversion) + DVE (colmax 2x + rowmax 4x).
        with loop_ctx:
          for _rep in range(reps):
            for it in range(IT_N):
                for g in range(NG):
                    P = ps.tile([128, GSPAN], F32)
                    for j in range(GRP):
                        nc.tensor.matmul(
                            P[:, j * 512 : (j + 1) * 512],
                            L[:, it * 128 : (it + 1) * 128],
                            R[:, (g * GRP + j) * 512 : (g * GRP + j + 1) * 512],
                            start=True,
                            stop=True,
                        )
                    src, dst = (cmb_a, cmb_b) if it % 2 == 0 else (cmb_b, cmb_a)
                    g0 = g * GSPAN
                    C = cbp.tile([128, GSPAN], BF16)
                    # PSUM->SBUF bf16 conversion split ACT/DVE by column
                    nc.scalar.copy(C[:, 0:CA], P[:, 0:CA])
                    if CA < GSPAN:
                        nc.vector.tensor_copy(C[:, CA:GSPAN], P[:, CA:GSPAN])
                    # colmax accumulate (bf16 tensor_tensor, 2x mode)
                    nc.vector.tensor_tensor(
                        dst[:, g0 : g0 + GSPAN], src[:, g0 : g0 + GSPAN], C[:], op=MX
                    )
                    # rowmax: ONE 4x tensor_scalar pass, accum_out reduces
                    # the row with op1=max
                    ts = tsp.tile([128, GSPAN], BF16)
                    nc.vector.tensor_scalar(
                        ts[:], C[:], -3.0e38, None, op0=MX, op1=MX,
                        accum_out=rg_all[:, it * NG + g : it * NG + g + 1],
                    )

        # fold the NG per-group rowmaxes into one column per I-tile
        rga_v = rg_all[:].rearrange("p (t g) -> p t g", g=NG)
        nc.vector.tensor_tensor(rmall[:], rga_v[:, :, 0], rga_v[:, :, 1], op=MX)
        for g in range(2, NG):
            nc.vector.tensor_tensor(rmall[:], rmall[:], rga_v[:, :, g], op=MX)

        # ---- tail: dump cmb raw in two column halves (the first only needs
        # the g=0 groups, so it overlaps the last tiles); the host does the
        # 128-way partition max plus clamp/sqrt/mean ----
        cmb_fin = cmb_b if (IT_N * reps) % 2 == 1 else cmb_a
        H = NPTS // 2
        nc.sync.dma_start(outc_d[:, 0:H], cmb_fin[:, 0:H])
        nc.sync.dma_start(outc_d[:, H:], cmb_fin[:, H:])
        nc.scalar.dma_start(outr_d, rmall[:])

    nc.compile()
    return nc


def _get(reps: int = 1, loop_n: int = 1, **kw):
    key = (reps, loop_n, tuple(sorted(kw.items())))
    if key not in _cached:
        _cached[key] = _build(reps, loop_n, **kw)
    return _cached[key]


def kernel(input1: np.ndarray, input2: np.ndarray, _trace: bool = False):
    nc = _get()
    input1 = np.ascontiguousarray(np.asarray(input1, dtype=np.float32))
    input2 = np.ascontiguousarray(np.asarray(input2, dtype=np.float32))
    in_maps = [{"x1": input1[b], "x2": input2[b]} for b in range(B)]
    res = run_bass_kernel_spmd(nc, in_maps, core_ids=list(range(B)), trace=_trace)
    losses = []
    for b in range(B):
        r = res.results[b]
        c = -np.asarray(r["outc"], dtype=np.float64).max(axis=0)
        rr = -np.asarray(r["outr"], dtype=np.float64).reshape(-1)
        d0 = np.sqrt(np.clip(c, 0.0, None))
        d1 = np.sqrt(np.clip(rr, 0.0, None))
        losses.append(d0.mean() + d1.mean())
    out = np.float32(np.mean(losses))
    if _trace:
        return out, res
    return out
